# revision 3
# baseline (speedup 1.0000x reference)
"""BERT self-attention (B=4, S=2048, H=768, 12 heads) on 8 trn2 NeuronCores.

Sharding: core c handles batch b=c//2 and heads h0=6*(c%2) .. h0+5.
Per core the kernel computes, for each of its 6 heads:
  scores = (hs @ (Wq/8).T + bq/8) @ (hs @ Wk.T + bk).T + mask     [2048, 2048]
  ctxT_aug = [V | 1].T-contracted with exp(scores.T)              [65, 2048]
where row 64 of ctxT_aug is the softmax denominator (sum_k exp).
The softmax division + head-concat transpose happen on the host.

Tricks:
  - Q is pre-scaled by 1/8 on the host (fold the 1/sqrt(64) into Wq, bq).
  - Q/K projection bias is added via the ScalarE activation bias port
    during the PSUM->SBUF copy; V bias via a K=1 matmul with a ones row.
  - The additive attention mask rides in contraction row 64 of the
    augmented Q^T/K^T operands (ones row x mask row), so both the [q,k]
    and the [k,q] score matmuls include it for free.
  - exp() runs on ScalarE straight out of PSUM, no max-subtraction
    (scores are O(1) here), writing fp32r tiles that feed the V matmul.
  - A ones column appended to V makes the same matmul accumulate the
    softmax denominator into row 64 of the context output.
All matmuls run as float32r (full PE rate at N>=256, ~1e-4 rel err).
"""

import numpy as np

B, S, H, NH, HD = 4, 2048, 768, 12, 64
NHC = 6            # heads per core
HDA = HD + 1       # head dim + denominator row
FT = H // 128      # 6 f-tiles of the contraction over hidden dim
KT = S // 128      # 16 seq tiles of 128
CH = S // 512      # 4 seq chunks of 512 (one PSUM bank each)

_CACHE = {}


def _build_nc():
    import concourse.mybir as mybir
    from concourse import bacc
    from concourse.tile import TileContext

    f32 = mybir.dt.float32
    f32r = mybir.dt.float32r
    IDENT = mybir.ActivationFunctionType.Identity
    EXP = mybir.ActivationFunctionType.Exp

    nc = bacc.Bacc(None, target_bir_lowering=False)
    hsT = nc.dram_tensor("hsT", [H, S], f32r, kind="ExternalInput")
    wqT = nc.dram_tensor("wqT", [H, NHC * HD], f32r, kind="ExternalInput")
    wkT = nc.dram_tensor("wkT", [H, NHC * HD], f32r, kind="ExternalInput")
    wvT = nc.dram_tensor("wvT", [H + 1, NHC * HD], f32r, kind="ExternalInput")
    bq2 = nc.dram_tensor("bq2", [HD, NHC], f32, kind="ExternalInput")
    bk2 = nc.dram_tensor("bk2", [HD, NHC], f32, kind="ExternalInput")
    mask = nc.dram_tensor("mask", [1, S], f32r, kind="ExternalInput")
    scores_o = nc.dram_tensor("scores_o", [NHC, S, S], f32, kind="ExternalOutput")
    ctxT_o = nc.dram_tensor("ctxT_o", [NHC, HDA, S], f32, kind="ExternalOutput")

    with TileContext(nc) as tc:
        with tc.tile_pool(name="persist", bufs=1) as P, \
             tc.tile_pool(name="qk", bufs=2) as QK, \
             tc.tile_pool(name="expp", bufs=2) as EX, \
             tc.tile_pool(name="sout", bufs=3) as SO, \
             tc.tile_pool(name="ctxs", bufs=1) as CS, \
             tc.tile_pool(name="psS", bufs=2, space="PSUM") as PS_S, \
             tc.tile_pool(name="psW", bufs=2, space="PSUM") as PS_W, \
             tc.tile_pool(name="psC", bufs=1, space="PSUM") as PS_C:

            # ---- persistent loads ----
            hsT_sb = []
            for i in range(FT):
                t = P.tile([128, S], f32r, tag=f"hsT{i}")
                nc.sync.dma_start(out=t, in_=hsT[i * 128:(i + 1) * 128, :])
                hsT_sb.append(t)
            wq_sb, wk_sb, wv_sb = [], [], []
            for i in range(FT):
                tq = P.tile([128, NHC * HD], f32r, tag=f"wq{i}")
                nc.sync.dma_start(out=tq, in_=wqT[i * 128:(i + 1) * 128, :])
                wq_sb.append(tq)
                tk = P.tile([128, NHC * HD], f32r, tag=f"wk{i}")
                nc.sync.dma_start(out=tk, in_=wkT[i * 128:(i + 1) * 128, :])
                wk_sb.append(tk)
                tv = P.tile([128, NHC * HD], f32r, tag=f"wv{i}")
                nc.sync.dma_start(out=tv, in_=wvT[i * 128:(i + 1) * 128, :])
                wv_sb.append(tv)
            wv_bias = P.tile([1, NHC * HD], f32r, tag="wvb")
            nc.sync.dma_start(out=wv_bias, in_=wvT[H:H + 1, :])
            bq_sb = P.tile([HD, NHC], f32, tag="bq")
            nc.sync.dma_start(out=bq_sb, in_=bq2[:, :])
            bk_sb = P.tile([HD, NHC], f32, tag="bk")
            nc.sync.dma_start(out=bk_sb, in_=bk2[:, :])
            ones1 = P.tile([1, 128], f32r, tag="ones1")
            nc.gpsimd.memset(ones1.bitcast(mybir.dt.float32), 1.0)

            # ---- V projection for all 6 heads: v_sb[:, kt, h, 0:64]=V, col 64=1
            v_sb = P.tile([128, KT, NHC, HDA], f32r, tag="v")
            for st in range(KT):
                pv = PS_W.tile([128, NHC * HD], f32, tag="w")
                for ft in range(FT):
                    nc.tensor.matmul(
                        pv, hsT_sb[ft][:, st * 128:(st + 1) * 128], wv_sb[ft],
                        start=(ft == 0), stop=False)
                nc.tensor.matmul(pv, ones1, wv_bias, start=False, stop=True)
                nc.vector.tensor_copy(
                    v_sb[:, st, :, 0:HD],
                    pv.rearrange("p (h d) -> p h d", h=NHC))
            nc.gpsimd.memset(v_sb[:, :, :, HD:HDA].bitcast(mybir.dt.float32), 1.0)

            # ---- per-head attention ----
            for h in range(NHC):
                hc = slice(h * HD, (h + 1) * HD)
                # augmented Q^T (rows 0..63 = Q^T/8, row 64 = ones)
                # augmented K^T (rows 0..63 = K^T, row 64 = mask)
                qaT = QK.tile([HDA, S], f32r, tag="qaT")
                kaT = QK.tile([HDA, S], f32r, tag="kaT")
                nc.gpsimd.memset(qaT[HD:HDA, :].bitcast(mybir.dt.float32), 1.0)
                nc.sync.dma_start(out=kaT[HD:HDA, :], in_=mask[:, :])
                for c in range(CH):
                    cs512 = slice(c * 512, (c + 1) * 512)
                    pq = PS_W.tile([HD, 512], f32, tag="w")
                    for ft in range(FT):
                        nc.tensor.matmul(
                            pq, wq_sb[ft][:, hc], hsT_sb[ft][:, cs512],
                            start=(ft == 0), stop=(ft == FT - 1))
                    nc.scalar.activation(qaT[0:HD, cs512], pq, IDENT,
                                         bias=bq_sb[:, h:h + 1], scale=1.0)
                    pk = PS_W.tile([HD, 512], f32, tag="w")
                    for ft in range(FT):
                        nc.tensor.matmul(
                            pk, wk_sb[ft][:, hc], hsT_sb[ft][:, cs512],
                            start=(ft == 0), stop=(ft == FT - 1))
                    nc.scalar.activation(kaT[0:HD, cs512], pk, IDENT,
                                         bias=bk_sb[:, h:h + 1], scale=1.0)

                ctx_ps = PS_C.tile([HDA, S], f32, tag="ctx")
                for t in range(KT):
                    ts128 = slice(t * 128, (t + 1) * 128)
                    # scores tile [q=t, all k] -> DRAM
                    so = SO.tile([128, S], f32, tag="so")
                    for c in range(CH):
                        cs512 = slice(c * 512, (c + 1) * 512)
                        ps = PS_S.tile([128, 512], f32, tag="s")
                        nc.tensor.matmul(ps, qaT[:, ts128], kaT[:, cs512],
                                         start=True, stop=True)
                        nc.vector.tensor_copy(so[:, cs512], ps)
                    nc.sync.dma_start(out=scores_o[h, ts128, :], in_=so)
                    # transposed scores tile [k=t, all q] -> exp -> ctx accum
                    ex = EX.tile([128, S], f32r, tag="ex")
                    for c in range(CH):
                        cs512 = slice(c * 512, (c + 1) * 512)
                        pst = PS_W.tile([128, 512], f32, tag="w")
                        nc.tensor.matmul(pst, kaT[:, ts128], qaT[:, cs512],
                                         start=True, stop=True)
                        nc.scalar.activation(ex[:, cs512], pst, EXP)
                    for c in range(CH):
                        cs512 = slice(c * 512, (c + 1) * 512)
                        nc.tensor.matmul(
                            ctx_ps[:, cs512], v_sb[:, t, h, :], ex[:, cs512],
                            start=(t == 0), stop=(t == KT - 1),
                            skip_group_check=True)
                ctxs = CS.tile([HDA, S], f32, tag="cs")
                nc.vector.tensor_copy(ctxs, ctx_ps)
                nc.sync.dma_start(out=ctxT_o[h, :, :], in_=ctxs)

    nc.compile()
    return nc


def _get_nc():
    if "nc" not in _CACHE:
        _CACHE["nc"] = _build_nc()
    return _CACHE["nc"]


def _prepare_in_maps(inputs):
    hs = np.ascontiguousarray(np.asarray(inputs["hidden_states"], dtype=np.float32))
    am = np.asarray(inputs["attention_mask"], dtype=np.float32)
    Wq = np.asarray(inputs["Wq"], dtype=np.float32)
    bq = np.asarray(inputs["bq"], dtype=np.float32)
    Wk = np.asarray(inputs["Wk"], dtype=np.float32)
    bk = np.asarray(inputs["bk"], dtype=np.float32)
    Wv = np.asarray(inputs["Wv"], dtype=np.float32)
    bv = np.asarray(inputs["bv"], dtype=np.float32)

    in_maps = []
    for c in range(8):
        b, h0 = c // 2, (c % 2) * NHC
        cols = slice(h0 * HD, (h0 + NHC) * HD)
        in_maps.append({
            "hsT": np.ascontiguousarray(hs[b].T),
            "wqT": np.ascontiguousarray(Wq.T[:, cols]) / 8.0,
            "wkT": np.ascontiguousarray(Wk.T[:, cols]),
            "wvT": np.ascontiguousarray(
                np.vstack([Wv.T[:, cols], bv[cols][None, :]])),
            "bq2": np.ascontiguousarray((bq[cols] / 8.0).reshape(NHC, HD).T),
            "bk2": np.ascontiguousarray(bk[cols].reshape(NHC, HD).T),
            "mask": np.ascontiguousarray(am[b, 0, 0, :][None, :]),
        })
    return in_maps


def _assemble_outputs(results):
    scores = np.empty((B, NH, S, S), dtype=np.float32)
    ctx = np.empty((B, S, H), dtype=np.float32)
    for c in range(8):
        b, h0 = c // 2, (c % 2) * NHC
        scores[b, h0:h0 + NHC] = results[c]["scores_o"]
        ct = results[c]["ctxT_o"]                  # [6, 65, 2048]
        un = ct[:, :HD, :] / ct[:, HD:HDA, :]      # softmax divide
        ctx[b, :, h0 * HD:(h0 + NHC) * HD] = (
            un.transpose(2, 0, 1).reshape(S, NHC * HD))
    return ctx, scores


def kernel(**inputs):
    from concourse.bass_utils import run_bass_kernel_spmd

    nc = _get_nc()
    in_maps = _prepare_in_maps(inputs)
    res = run_bass_kernel_spmd(nc, in_maps, core_ids=list(range(8)))
    _CACHE["last_result"] = res
    return _assemble_outputs(res.results)


# revision 10
# speedup vs baseline: 1.5940x; 1.5940x over previous
"""BERT self-attention (B=4, S=2048, H=768, 12 heads) on 8 trn2 NeuronCores.

Sharding: core c handles batch b=c//2 and heads h0=6*(c%2) .. h0+5.
Per core the kernel computes, for each of its 6 heads:
  scores = (hs @ (Wq/8).T + bq/8) @ (hs @ Wk.T + bk).T + mask     [2048, 2048]
  ctxT_aug = [V | 1].T-contracted with exp(scores.T)              [65, 2048]
where row 64 of ctxT_aug is the softmax denominator (sum_k exp).
The softmax division + head-concat transpose happen on the host.

Tricks:
  - Q is pre-scaled by 1/8 on the host (fold the 1/sqrt(64) into Wq, bq).
  - Q/K projection bias is added via the ScalarE activation bias port
    during the PSUM->SBUF copy; V bias via a K=1 matmul with a ones row.
  - The additive attention mask rides in contraction row 64 of the
    augmented Q^T/K^T operands (ones row x mask row), so both the [q,k]
    and the [k,q] score matmuls include it for free.
  - exp() runs on ScalarE straight out of PSUM, no max-subtraction
    (scores are O(1) here), writing fp32r tiles that feed the V matmul.
  - A ones column appended to V makes the same matmul accumulate the
    softmax denominator into row 64 of the context output.
All matmuls run as float32r (full PE rate at N>=256, ~1e-4 rel err).
"""

import numpy as np

B, S, H, NH, HD = 4, 2048, 768, 12, 64
NHC = 6            # heads per core
HDA = HD + 1       # head dim + denominator row
FT = H // 128      # 6 f-tiles of the contraction over hidden dim
KT = S // 128      # 16 seq tiles of 128
CH = S // 512      # 4 seq chunks of 512 (one PSUM bank each)

_CACHE = {}


def _build_nc():
    import concourse.mybir as mybir
    from concourse import bacc
    from concourse.tile import TileContext

    f32 = mybir.dt.float32
    f32r = mybir.dt.float32r
    IDENT = mybir.ActivationFunctionType.Identity
    EXP = mybir.ActivationFunctionType.Exp

    nc = bacc.Bacc(None, target_bir_lowering=False)
    hsT = nc.dram_tensor("hsT", [H, S], f32r, kind="ExternalInput")
    wqT = nc.dram_tensor("wqT", [H, NHC * HD], f32r, kind="ExternalInput")
    wkT = nc.dram_tensor("wkT", [H, NHC * HD], f32r, kind="ExternalInput")
    wvT = nc.dram_tensor("wvT", [H + 1, NHC * HD], f32r, kind="ExternalInput")
    bq2 = nc.dram_tensor("bq2", [HD, NHC], f32, kind="ExternalInput")
    bk2 = nc.dram_tensor("bk2", [HD, NHC], f32, kind="ExternalInput")
    mask = nc.dram_tensor("mask", [1, S], f32r, kind="ExternalInput")
    scores_o = nc.dram_tensor("scores_o", [NHC, S, S], f32, kind="ExternalOutput")
    ctxT_o = nc.dram_tensor("ctxT_o", [NHC, HDA, S], f32, kind="ExternalOutput")

    with TileContext(nc) as tc:
        with tc.tile_pool(name="persist", bufs=1) as P, \
             tc.tile_pool(name="qk", bufs=2) as QK, \
             tc.tile_pool(name="expp", bufs=3) as EX, \
             tc.tile_pool(name="sout", bufs=3) as SO, \
             tc.tile_pool(name="ctxs", bufs=1) as CS, \
             tc.tile_pool(name="psS", bufs=2, space="PSUM") as PS_S, \
             tc.tile_pool(name="psW", bufs=2, space="PSUM") as PS_W, \
             tc.tile_pool(name="psC", bufs=1, space="PSUM") as PS_C:

            # ---- persistent loads (DMA order = SP FIFO order: interleave so
            # the first Q/K projection matmuls can start after ~one f-tile) ----
            hsT_sb, wq_sb, wk_sb = [], [], []
            for i in range(FT):
                t = P.tile([128, S], f32r, tag=f"hsT{i}")
                nc.sync.dma_start(out=t, in_=hsT[i * 128:(i + 1) * 128, :])
                hsT_sb.append(t)
                tq = P.tile([128, NHC * HD], f32r, tag=f"wq{i}")
                nc.sync.dma_start(out=tq, in_=wqT[i * 128:(i + 1) * 128, :])
                wq_sb.append(tq)
                tk = P.tile([128, NHC * HD], f32r, tag=f"wk{i}")
                nc.sync.dma_start(out=tk, in_=wkT[i * 128:(i + 1) * 128, :])
                wk_sb.append(tk)
            bq_sb = P.tile([HD, NHC], f32, tag="bq")
            nc.sync.dma_start(out=bq_sb, in_=bq2[:, :])
            bk_sb = P.tile([HD, NHC], f32, tag="bk")
            nc.sync.dma_start(out=bk_sb, in_=bk2[:, :])
            wv_sb = []
            for i in range(FT):
                tv = P.tile([128, NHC * HD], f32r, tag=f"wv{i}")
                nc.sync.dma_start(out=tv, in_=wvT[i * 128:(i + 1) * 128, :])
                wv_sb.append(tv)
            wv_bias = P.tile([1, NHC * HD], f32r, tag="wvb")
            nc.sync.dma_start(out=wv_bias, in_=wvT[H:H + 1, :])
            ones1 = P.tile([1, 128], f32r, tag="ones1")
            nc.gpsimd.memset(ones1.bitcast(mybir.dt.float32), 1.0)

            # V for all 6 heads: v_sb[:, kt, h, 0:64]=V, col 64=1 (sums trick).
            # The 16 projection tiles are emitted inside head 0's t-loop so
            # they overlap the first scores tiles instead of blocking them.
            v_sb = P.tile([128, KT, NHC, HDA], f32r, tag="v")
            nc.gpsimd.memset(v_sb[:, :, :, HD:HDA].bitcast(mybir.dt.float32), 1.0)

            def emit_vproj(st):
                pv = PS_W.tile([128, NHC * HD], f32, tag="w")
                for ft in range(FT):
                    nc.tensor.matmul(
                        pv, hsT_sb[ft][:, st * 128:(st + 1) * 128], wv_sb[ft],
                        start=(ft == 0), stop=False)
                nc.tensor.matmul(pv, ones1, wv_bias, start=False, stop=True)
                nc.vector.tensor_copy(
                    v_sb[:, st, :, 0:HD],
                    pv.rearrange("p (h d) -> p h d", h=NHC))

            # ---- per-head attention ----
            for h in range(NHC):
                hc = slice(h * HD, (h + 1) * HD)
                # augmented Q^T (rows 0..63 = Q^T/8, row 64 = ones)
                # augmented K^T (rows 0..63 = K^T, row 64 = mask)
                qaT = QK.tile([HDA, S], f32r, tag="qaT")
                kaT = QK.tile([HDA, S], f32r, tag="kaT")
                nc.gpsimd.memset(qaT[HD:HDA, :].bitcast(mybir.dt.float32), 1.0)
                nc.sync.dma_start(out=kaT[HD:HDA, :], in_=mask[:, :])
                for c in range(CH):
                    cs512 = slice(c * 512, (c + 1) * 512)
                    pq = PS_W.tile([HD, 512], f32, tag="w")
                    for ft in range(FT):
                        nc.tensor.matmul(
                            pq, wq_sb[ft][:, hc], hsT_sb[ft][:, cs512],
                            start=(ft == 0), stop=(ft == FT - 1))
                    nc.scalar.activation(qaT[0:HD, cs512], pq, IDENT,
                                         bias=bq_sb[:, h:h + 1], scale=1.0)
                    pk = PS_W.tile([HD, 512], f32, tag="w")
                    for ft in range(FT):
                        nc.tensor.matmul(
                            pk, wk_sb[ft][:, hc], hsT_sb[ft][:, cs512],
                            start=(ft == 0), stop=(ft == FT - 1))
                    nc.scalar.activation(kaT[0:HD, cs512], pk, IDENT,
                                         bias=bk_sb[:, h:h + 1], scale=1.0)

                ctx_ps = PS_C.tile([HDA, S], f32, tag="ctx")
                for t in range(KT):
                    ts128 = slice(t * 128, (t + 1) * 128)
                    if h == 0:
                        emit_vproj(t)
                    # scores tile [q=t, all k] -> DRAM
                    so = SO.tile([128, S], f32, tag="so")
                    for c in range(CH):
                        cs512 = slice(c * 512, (c + 1) * 512)
                        ps = PS_S.tile([128, 512], f32, tag="s")
                        nc.tensor.matmul(ps, qaT[:, ts128], kaT[:, cs512],
                                         start=True, stop=True)
                        nc.vector.tensor_copy(so[:, cs512], ps)
                    nc.sync.dma_start(out=scores_o[h, ts128, :], in_=so)
                    # transposed scores tile [k=t, all q] -> exp -> ctx accum
                    ex = EX.tile([128, S], f32r, tag="ex")
                    for c in range(CH):
                        cs512 = slice(c * 512, (c + 1) * 512)
                        pst = PS_W.tile([128, 512], f32, tag="w")
                        nc.tensor.matmul(pst, kaT[:, ts128], qaT[:, cs512],
                                         start=True, stop=True)
                        nc.scalar.activation(ex[:, cs512], pst, EXP)
                    for c in range(CH):
                        cs512 = slice(c * 512, (c + 1) * 512)
                        nc.tensor.matmul(
                            ctx_ps[:, cs512], v_sb[:, t, h, :], ex[:, cs512],
                            start=(t == 0), stop=(t == KT - 1),
                            skip_group_check=True)
                ctxs = CS.tile([HDA, S], f32, tag="cs")
                nc.vector.tensor_copy(ctxs, ctx_ps)
                nc.sync.dma_start(out=ctxT_o[h, :, :], in_=ctxs)

    nc.compile()
    return nc


def _get_nc():
    if "nc" not in _CACHE:
        _CACHE["nc"] = _build_nc()
    return _CACHE["nc"]


def _prepare_in_maps(inputs):
    hs = np.ascontiguousarray(np.asarray(inputs["hidden_states"], dtype=np.float32))
    am = np.asarray(inputs["attention_mask"], dtype=np.float32)
    Wq = np.asarray(inputs["Wq"], dtype=np.float32)
    bq = np.asarray(inputs["bq"], dtype=np.float32)
    Wk = np.asarray(inputs["Wk"], dtype=np.float32)
    bk = np.asarray(inputs["bk"], dtype=np.float32)
    Wv = np.asarray(inputs["Wv"], dtype=np.float32)
    bv = np.asarray(inputs["bv"], dtype=np.float32)

    in_maps = []
    for c in range(8):
        b, h0 = c // 2, (c % 2) * NHC
        cols = slice(h0 * HD, (h0 + NHC) * HD)
        in_maps.append({
            "hsT": np.ascontiguousarray(hs[b].T),
            "wqT": np.ascontiguousarray(Wq.T[:, cols]) / 8.0,
            "wkT": np.ascontiguousarray(Wk.T[:, cols]),
            "wvT": np.ascontiguousarray(
                np.vstack([Wv.T[:, cols], bv[cols][None, :]])),
            "bq2": np.ascontiguousarray((bq[cols] / 8.0).reshape(NHC, HD).T),
            "bk2": np.ascontiguousarray(bk[cols].reshape(NHC, HD).T),
            "mask": np.ascontiguousarray(am[b, 0, 0, :][None, :]),
        })
    return in_maps


def _assemble_outputs(results):
    scores = np.empty((B, NH, S, S), dtype=np.float32)
    ctx = np.empty((B, S, H), dtype=np.float32)
    for c in range(8):
        b, h0 = c // 2, (c % 2) * NHC
        scores[b, h0:h0 + NHC] = results[c]["scores_o"]
        ct = results[c]["ctxT_o"]                  # [6, 65, 2048]
        un = ct[:, :HD, :] / ct[:, HD:HDA, :]      # softmax divide
        ctx[b, :, h0 * HD:(h0 + NHC) * HD] = (
            un.transpose(2, 0, 1).reshape(S, NHC * HD))
    return ctx, scores


def kernel(**inputs):
    from concourse.bass_utils import run_bass_kernel_spmd

    nc = _get_nc()
    in_maps = _prepare_in_maps(inputs)
    res = run_bass_kernel_spmd(nc, in_maps, core_ids=list(range(8)))
    _CACHE["last_result"] = res
    return _assemble_outputs(res.results)
